# revision 13
# baseline (speedup 1.0000x reference)
"""AdaptiveuBCLLoss on 8 TRN2 NeuronCores.

loss = mean_i log sum_j exp(lambda * (cos(z1_i, z2_j) - cos(z1_i, z2_i)))
with z1 = output[:, 0], z2 = output[:, 1], N=4096, D=1024.

Sharding: rows of z1 are split 512/core. Each core receives:
  - z1t  [1024, 512]: its z1 slab, transposed (matmul lhsT layout)
  - z1r  [512, 1024]: its z1 slab, row layout (for row norms)
  - z2t  [1024, 4096]: full z2 transposed, columns ROTATED by 512*c so the
    diagonal block of the cosine matrix always lands in column group 0.
    Row-wise log-sum-exp is invariant to the column permutation, so every
    core runs the identical SPMD graph with no core-id input.
  - lam  [1, 1], eye [128, 128] constants.
Output per core: out [512] = per-row log-sum-exp. Host: mean of all 4096.
"""

import numpy as np

import concourse.bass as bass
import concourse.bacc as bacc
import concourse.tile as tile
import concourse.mybir as mybir
from concourse.bass_utils import run_bass_kernel_spmd

N = 4096
D = 1024
NCORES = 8
RPC = N // NCORES  # 512 rows per core
P = 128
RT = RPC // P      # 4 row tiles per core
NG = N // 512      # 8 column groups of 512
KC = D // P        # 8 contraction chunks of 128

F32 = mybir.dt.float32
F32R = mybir.dt.float32r
BF16 = mybir.dt.bfloat16
AF = mybir.ActivationFunctionType
AX = mybir.AxisListType


def build_nc():
    nc = bacc.Bacc("TRN2", target_bir_lowering=False, debug=False, num_devices=NCORES)

    # z1t/z2t feed the TensorEngine in float32r (relaxed-precision fp32,
    # 1 cycle/row vs 4 for plain fp32). Same bytes as f32 host-side.
    z1t_d = nc.dram_tensor("z1t", [D, RPC], F32R, kind="ExternalInput").ap()
    z1r_d = nc.dram_tensor("z1r", [RPC, D], F32, kind="ExternalInput").ap()
    z2t_d = nc.dram_tensor("z2t", [D, N], F32R, kind="ExternalInput").ap()
    lam_d = nc.dram_tensor("lam", [1, 1], F32, kind="ExternalInput").ap()
    eye_d = nc.dram_tensor("eye", [P, P], F32, kind="ExternalInput").ap()
    out_d = nc.dram_tensor("out", [RPC], F32, kind="ExternalOutput").ap()

    with tile.TileContext(nc) as tc:
        with (
            tc.tile_pool(name="persist", bufs=1) as persist,
            tc.tile_pool(name="sq", bufs=3) as sqp,
            tc.tile_pool(name="ghat", bufs=3) as ghatp,
            tc.tile_pool(name="etile", bufs=2) as ep,
            tc.tile_pool(name="small", bufs=4) as smallp,
            tc.tile_pool(name="n2p", bufs=2) as n2p,
            tc.tile_pool(name="gps", bufs=4, space="PSUM") as gps,
            tc.tile_pool(name="nps", bufs=2, space="PSUM") as nps,
        ):
            # ---- persistent SBUF tensors ----
            z1t_sb = persist.tile([P, KC, RPC], F32R)      # [p, k, i] = z1t[128k+p, i]
            z1r_sb = persist.tile([P, RT, D], F32)         # [p, t, d] = z1[128t+p, d]
            z2t_sb = persist.tile([P, NG, KC, 512], F32R)  # [p, g, k, n] = z2t[128k+p, 512g+n]
            r2_sb = persist.tile([P, N], F32)              # 1/||z2_j|| bcast over partitions
            eye_sb = persist.tile([P, P], F32)
            ones_sb = persist.tile([P, P], BF16)
            lam_sb = persist.tile([P, 1], F32)
            eps_sb = persist.tile([P, 1], F32)
            s_sb = persist.tile([P, RT, NG], F32)          # exp row partial sums
            lse_sb = persist.tile([P, RT], F32)            # final lse rows

            # ---- input DMAs ----
            nc.sync.dma_start(out=lam_sb, in_=lam_d.to_broadcast((P, 1)))
            nc.sync.dma_start(out=eye_sb, in_=eye_d)
            nc.sync.dma_start(
                out=z1t_sb, in_=z1t_d.rearrange("(k p) i -> p k i", p=P)
            )
            nc.sync.dma_start(
                out=z1r_sb, in_=z1r_d.rearrange("(t p) d -> p t d", p=P)
            )
            for g in range(NG):
                nc.sync.dma_start(
                    out=z2t_sb[:, g],
                    in_=z2t_d[:, g * 512 : (g + 1) * 512].rearrange(
                        "(k p) n -> p k n", p=P
                    ),
                )

            nc.vector.memset(ones_sb, 1.0)
            nc.vector.memset(eps_sb, 1e-16)

            # ---- r1: per-partition 1/||z1_i|| for each row tile ----
            r1 = []
            negl_r1 = []  # -lambda * r1
            lam_r1 = []   # +lambda * r1
            sq_scratch = persist.tile([P, D], F32)
            for t in range(RT):
                n1sq = smallp.tile([P, 1], F32, name="n1sq")
                nc.scalar.activation(
                    out=sq_scratch,
                    in_=z1r_sb[:, t],
                    func=AF.Square,
                    accum_out=n1sq,
                )
                n1 = smallp.tile([P, 1], F32, name="n1")
                nc.scalar.activation(out=n1, in_=n1sq, func=AF.Sqrt, bias=eps_sb)
                r1_t = persist.tile([P, 1], F32, name=f"r1_{t}")
                nc.vector.reciprocal(out=r1_t, in_=n1)
                lam_r1_t = persist.tile([P, 1], F32, name=f"lamr1_{t}")
                nc.vector.tensor_mul(out=lam_r1_t, in0=r1_t, in1=lam_sb)
                negl_r1_t = persist.tile([P, 1], F32, name=f"neglr1_{t}")
                nc.vector.tensor_scalar_mul(out=negl_r1_t, in0=lam_r1_t, scalar1=-1.0)
                r1.append(r1_t)
                lam_r1.append(lam_r1_t)
                negl_r1.append(negl_r1_t)

            bias_t = [None] * RT  # -lambda*r1*pos, filled at g==0

            # ---- main loop: group-outer to pipeline behind z2t DMA ----
            for g in range(NG):
                # column norms for this group: n2sq (broadcast across partitions)
                # via ones-matmul over squared z2t chunks
                n2sq_ps = nps.tile([P, 512], F32, name="n2sq")
                for k in range(KC):
                    sq = sqp.tile([P, 512], BF16, name="sq")
                    src = z2t_sb[:, g, k].bitcast(F32)
                    if k % 2 == 0:
                        nc.vector.tensor_mul(out=sq, in0=src, in1=src)
                    else:
                        nc.scalar.activation(out=sq, in_=src, func=AF.Square)
                    nc.tensor.matmul(
                        n2sq_ps,
                        ones_sb,
                        sq,
                        start=(k == 0),
                        stop=(k == KC - 1),
                    )
                n2 = n2p.tile([P, 512], F32, name="n2f")
                nc.scalar.activation(out=n2, in_=n2sq_ps, func=AF.Sqrt, bias=eps_sb)
                nc.vector.reciprocal(out=r2_sb[:, g * 512 : (g + 1) * 512], in_=n2)

                for t in range(RT):
                    g_ps = gps.tile([P, 512], F32, name="g_ps")
                    for k in range(KC):
                        nc.tensor.matmul(
                            g_ps,
                            z1t_sb[:, k, t * P : (t + 1) * P],
                            z2t_sb[:, g, k],
                            start=(k == 0),
                            stop=(k == KC - 1),
                        )
                    # Ghat = G * r2 (column scale)
                    ghat = ghatp.tile([P, 512], F32, name="ghat")
                    nc.vector.tensor_mul(
                        out=ghat, in0=g_ps, in1=r2_sb[:, g * 512 : (g + 1) * 512]
                    )
                    if g == 0:
                        # extract pos (diagonal) via eye mask; diag block of
                        # row tile t sits at columns [128t : 128t+128]
                        dmask = smallp.tile([P, P], F32, name="dmask")
                        nc.vector.tensor_mul(
                            out=dmask,
                            in0=ghat[:, t * P : (t + 1) * P],
                            in1=eye_sb,
                        )
                        pos = smallp.tile([P, 1], F32, name="pos")
                        nc.vector.reduce_sum(out=pos, in_=dmask, axis=AX.X)
                        b = persist.tile([P, 1], F32, name=f"bias_{t}")
                        nc.vector.tensor_mul(out=b, in0=pos, in1=negl_r1[t])
                        bias_t[t] = b
                    # exp(lam*r1*ghat - lam*r1*pos), row-sum into s_sb[:, t, g]
                    etile = ep.tile([P, 512], F32, name="etile")
                    nc.scalar.activation(
                        out=etile,
                        in_=ghat,
                        func=AF.Exp,
                        bias=bias_t[t],
                        scale=lam_r1[t],
                        accum_out=s_sb[:, t, g : g + 1],
                    )

            # ---- finalize: lse rows, DMA out ----
            for t in range(RT):
                rowsum = smallp.tile([P, 1], F32, name="rowsum")
                nc.vector.reduce_sum(out=rowsum, in_=s_sb[:, t], axis=AX.X)
                nc.scalar.activation(
                    out=lse_sb[:, t : t + 1], in_=rowsum, func=AF.Ln
                )
            nc.sync.dma_start(
                out=out_d.rearrange("(t p) -> p t", p=P), in_=lse_sb
            )

    nc.compile()
    return nc


_NC_CACHE = None


def _get_nc():
    global _NC_CACHE
    if _NC_CACHE is None:
        _NC_CACHE = build_nc()
    return _NC_CACHE


def make_in_maps(output, lambda_):
    z1 = np.ascontiguousarray(output[:, 0]).astype(np.float32, copy=False)
    z2 = np.ascontiguousarray(output[:, 1]).astype(np.float32, copy=False)
    z2t = np.ascontiguousarray(z2.T)  # [D, N]
    lam = np.asarray(lambda_, dtype=np.float32).reshape(1, 1)
    eye = np.eye(P, dtype=np.float32)

    in_maps = []
    for c in range(NCORES):
        sl = slice(c * RPC, (c + 1) * RPC)
        z1r_c = np.ascontiguousarray(z1[sl])            # [512, 1024]
        z1t_c = np.ascontiguousarray(z1r_c.T)           # [1024, 512]
        z2t_c = np.ascontiguousarray(np.roll(z2t, -512 * c, axis=1))
        in_maps.append(
            {"z1t": z1t_c, "z1r": z1r_c, "z2t": z2t_c, "lam": lam, "eye": eye}
        )
    return in_maps


def kernel(output, lambda_):
    nc = _get_nc()
    in_maps = make_in_maps(output, lambda_)
    res = run_bass_kernel_spmd(nc, in_maps, core_ids=list(range(NCORES)))
    lse = np.concatenate([res.results[c]["out"] for c in range(NCORES)])
    return np.float32(lse.mean())


if __name__ == "__main__":
    rng = np.random.default_rng(0)
    output = rng.standard_normal((N, 2, D), dtype=np.float32)
    lambda_ = np.full((1,), 10.0, dtype=np.float32)
    got = kernel(output, lambda_)

    z1 = output[:, 0]
    z2 = output[:, 1]
    n1 = np.maximum(np.linalg.norm(z1, axis=-1, keepdims=True), 1e-8)
    n2 = np.maximum(np.linalg.norm(z2, axis=-1, keepdims=True), 1e-8)
    cos = (z1 / n1) @ (z2 / n2).T
    pos = np.diagonal(cos)[:, None]
    want = np.log(np.sum(np.exp(10.0 * (cos - pos)), axis=1)).mean()
    print("got", got, "want", want, "rel", abs(got - want) / abs(want))
